# revision 16
# baseline (speedup 1.0000x reference)
"""Attention2D Trainium2 Bass kernel — fp8 DoubleRow edition.

Reference computation (per batch image, C=512 channels, N=1024 tokens):
    qkv = qkv_w @ x + qkv_b            # (1536, N)
    q,k,v per head (8 heads, head_dim 64)
    attn = softmax(scale * q.T k)      # (N, N) per head, scale = C**-0.5
    out  = v @ attn.T                  # (64, N) per head
    y    = x + proj_w @ out + proj_b

Sharding: data-parallel over batch. 16 images / 8 cores = 2 images per core.
Weights replicated; no collectives.

Numerics/performance strategy (validated offline: rel err ~2.6e-3 vs the
2e-2 gate):
  - Everything quantized to fp8 e4m3. All the big channel-contraction
    matmuls (qkv, V, attn@V, proj) run in MatmulPerfMode.DoubleRow: one
    instruction contracts 2 k-tiles of 128 at 0.5 cycles/col (4x f32r).
    Layout for DR: lhsT [K,2,M], rhs [K,2,N] with the k-tile pair packed
    in the free dim.
  - S = q.T k has contraction 64 only, stays fp8 non-DR (1 cyc/col) with
    the head pair packed on partition halves (base partition 0/64).
  - K bias is dropped entirely: softmax(q.(k~+bk)) == softmax(q.k~ + f(n));
    the per-n term cancels. Q keeps its bias (supplies the bq.k~ term).
  - V bias folded into the residual on the host: sum_m attn = 1, so
    out_att = Vnorm + bv and y gains the constant proj_w@bv, merged into
    xr = x + proj_b + proj_w@bv.
  - exp is the elementwise bottleneck (2*8*1024^2 elems/core); only ACT
    and DVE can read PSUM. Split: ACT runs exact Exp (also absorbs the
    Q/K/V psum->sbuf fp8 conversions; 'exp_and_others' table covers Exp+
    Copy+Identity so no table reloads); DVE runs a Schraudolph-style
    integer exp: i8 = (S*scale*8*log2e) + (56.5-0.35), bitcast int8 ->
    fp8e4m3 (bias 7, 3 mantissa bits). Rounding-mode miscalibration is
    common-mode and cancels through the softmax denominator.
  - Softmax denominator comes free from a ones-column in the V^T tiles
    (row 64 of the attn@V psum). reciprocal on DVE, partition-broadcast
    on GPSIMD (sbuf->sbuf, no DRAM round trip), normalize-mul on DVE.
"""

import math

import numpy as np
import ml_dtypes

import concourse.bass as bass
import concourse.tile as tile
from concourse import mybir
from concourse.bass_utils import run_bass_kernel_spmd

B, C, N = 16, 512, 1024
HEADS, HD = 8, 64
SCALE = float(C) ** -0.5
NCORES = 8
BPC = B // NCORES  # images per core

F32 = mybir.dt.float32
F8 = mybir.dt.float8e4
BF16 = mybir.dt.bfloat16
I8 = mybir.dt.int8

# 'div_psum': tensor_scalar divide with the denominator column read straight
# from PSUM; 'rcp_mult': copy D -> sbuf, reciprocal [128,8], per-chunk mults.
NORM_MODE = "rcp_mult"

# Schraudolph exp -> fp8e4m3 constants: byte = round(x*SCALE*8*log2(e) + 56.5
# - 0.35). +56 = bias 7 << 3; +0.5 turns trunc into round; -0.35 recenters the
# piecewise-linear 2^frac error.
EXP_C1 = SCALE * 8.0 / math.log(2.0)
EXP_C2 = 56.5 - 0.35

DR = mybir.MatmulPerfMode.DoubleRow


def _split_multi_waits(nc):
    """Walrus codegen rejects instructions carrying more than one semaphore
    wait. Hoist all but the last wait into standalone InstEventSemaphore ops
    just before them (same engine, so per-engine order is preserved)."""
    n_split = 0
    for f in nc.m.functions:
        for b in f.blocks:
            out = []
            changed = False
            for inst in b.instructions:
                si = inst.sync_info
                waits = list(si.on_wait) if si is not None else []
                if len(waits) > 1:
                    for k, w in enumerate(waits[:-1]):
                        wi = mybir.InstEventSemaphore(
                            name=f"{inst.name}-presync{k}", ins=[], outs=[],
                            sync_info=mybir.SyncInfo(on_wait=[w], on_update=[]),
                        )
                        wi.engine = inst.engine
                        out.append(wi)
                        n_split += 1
                    inst.sync_info = mybir.SyncInfo(
                        on_wait=[waits[-1]], on_update=list(si.on_update)
                    )
                    changed = True
                out.append(inst)
            if changed:
                b.instructions = out
    return n_split


# Per-(phase, head-position) exp engine split: list of 8 engines for the 8
# m-chunks of a head. 'a' = ACT exact exp, 'd' = DVE integer exp.
# Phase 0 (image-0 heads): ACT also absorbs image-1 qkv/V conversions.
# Phase 1 (image-1 heads): DVE also absorbs proj residual adds.
_PAT_5A3D = ['a', 'd', 'a', 'a', 'd', 'a', 'd', 'a']
_PAT_4A4D = ['a', 'd', 'a', 'd', 'a', 'd', 'a', 'd']
_PAT_6A2D = ['a', 'a', 'd', 'a', 'a', 'd', 'a', 'a']
_PAT_7A1D = ['a', 'a', 'a', 'd', 'a', 'a', 'a', 'a']
_PAT_8A0D = ['a'] * 8
EXP_PAT = {
    0: [_PAT_4A4D, _PAT_5A3D] * 4,
    # tail is DVE-bound (proj adds + last norms): hand the late exps to ACT
    1: [_PAT_4A4D, _PAT_4A4D, _PAT_4A4D, _PAT_5A3D,
        _PAT_5A3D, _PAT_6A2D, _PAT_8A0D, _PAT_8A0D],
}


def build_nc():
    nc = bass.Bass()
    xm_h = nc.dram_tensor("xm", [BPC, C, N], F8, kind="ExternalInput")
    xr_h = nc.dram_tensor("xr", [BPC, C, N], F32, kind="ExternalInput")
    wqkv_h = nc.dram_tensor("wqkv", [C, 3 * C], F8, kind="ExternalInput")
    pw_h = nc.dram_tensor("pw", [C, C], BF16, kind="ExternalInput")
    bq_h = nc.dram_tensor("bq", [128, 4], F32, kind="ExternalInput")
    y_h = nc.dram_tensor("y", [BPC, C, N], F32, kind="ExternalOutput")

    MC = N // 128          # m-chunks (key/value token chunks)
    NH = N // 512          # moving-dim halves
    dma = nc.gpsimd.dma_start

    BUFS = dict(
        xm=4, xr=8, qk=18, vt=9, es=10, onT=18, tr=66, dd=3, y=3,
    )

    with tile.TileContext(nc) as tc:
        with (
            tc.tile_pool(name="w", bufs=1) as wp,
            tc.tile_pool(name="sb", bufs=2) as sb,
            tc.tile_pool(name="ps", bufs=2, space=bass.MemorySpace.PSUM) as ps,
            tc.tile_pool(name="pso", bufs=2, space=bass.MemorySpace.PSUM) as pso,
            tc.tile_pool(name="dr", bufs=4, space=bass.MemorySpace.DRAM) as dr,
        ):
            wqkv_dr = []   # u -> [128, 2*3C] fp8 (k-tile pair packed)
            pw_dr = []     # u -> [128, 2*C] fp8
            xm_dr = {}     # (img, u) -> [128, 2*N] fp8
            xr_sb = {}     # (img, oc) -> [128, N] f32
            qk_sb = {}     # (img, oc) -> [128, N] fp8 (oc 0-3 Q, 4-7 K)
            vt_sb = {}     # (img, j) -> [128, 2*8*65] fp8 (V^T pair + ones col)
            onT = {}       # (img, pair, a) -> [128, 128] bf16 O^T chunk
            tr = {}        # (img, cc, a) -> [128, 128] bf16 attn out (c, n)

            def wq_r(u):
                return wqkv_dr[u][:].rearrange("p (i o) -> p i o", i=2)

            def xm_r(img, u):
                return xm_dr[(img, u)][:].rearrange("p (i n) -> p i n", i=2)

            def load_weights():
                # weights on the gpsimd queue, image-0 x on the sync queue,
                # interleaved in first-use order so the first qkv matmul
                # unblocks after a couple of transfers per queue
                for u in range(2):
                    w = wp.tile([128, 2 * 3 * C], F8, tag=f"wqkv{u}",
                                name=f"wqkv{u}")
                    wqkv_dr.append(w)
                    t = sb.tile([128, 2 * N], F8, tag="xm", bufs=BUFS["xm"],
                                name=f"xm0_{u}")
                    xm_dr[(0, u)] = t
                    for i in range(2):
                        dma(out=w[:, i * 3 * C:(i + 1) * 3 * C],
                            in_=wqkv_h[(2 * u + i) * 128:(2 * u + i + 1) * 128, :])
                    if u == 0:
                        bq = wp.tile([128, 4], F32, tag="bq", name="bq")
                        dma(out=bq[:], in_=bq_h[:])
                    for nh in range(NH):
                        for i in range(2):
                            nc.sync.dma_start(
                                out=t[:, i * N + nh * 512:i * N + (nh + 1) * 512],
                                in_=xm_h[0, (2 * u + i) * 128:(2 * u + i + 1) * 128,
                                         nh * 512:(nh + 1) * 512])
                for cc in range(4):
                    w = wp.tile([128, C], BF16, tag=f"pw{cc}", name=f"pw{cc}")
                    pw_dr.append(w)
                    dma(out=w[:], in_=pw_h[cc * 128:(cc + 1) * 128, :])
                return bq

            def load_xm(img):
                for u in range(2):
                    t = sb.tile([128, 2 * N], F8, tag="xm", bufs=BUFS["xm"],
                                name=f"xm{img}_{u}")
                    for i in range(2):
                        for nh in range(NH):
                            dma(out=t[:, i * N + nh * 512:i * N + (nh + 1) * 512],
                                in_=xm_h[img, (2 * u + i) * 128:(2 * u + i + 1) * 128,
                                         nh * 512:(nh + 1) * 512])
                    xm_dr[(img, u)] = t

            def load_xr(img):
                for oc in range(4):
                    t = sb.tile([128, N], F32, tag="xr", bufs=BUFS["xr"],
                                name=f"xr{img}_{oc}")
                    nc.sync.dma_start(out=t[:], in_=xr_h[img, oc * 128:(oc + 1) * 128, :])
                    xr_sb[(img, oc)] = t

            def emit_qkv(img, ocs):
                # oc 0-3: Q chunks (bias added); oc 4-7: K chunks (no bias).
                for oc in ocs:
                    q_ps = ps.tile([128, N], F32, tag="s")
                    for nh in range(NH):
                        for u in range(2):
                            nc.tensor.matmul(
                                q_ps[:, nh * 512:(nh + 1) * 512],
                                wq_r(u)[:, :, oc * 128:(oc + 1) * 128],
                                xm_r(img, u)[:, :, nh * 512:(nh + 1) * 512],
                                start=(u == 0), stop=(u == 1), perf_mode=DR,
                            )
                    t = sb.tile([128, N], F8, tag="qk", bufs=BUFS["qk"],
                                name=f"qk{img}_{oc}")
                    if oc < 4:
                        nc.scalar.activation(
                            t[:], q_ps[:], mybir.ActivationFunctionType.Identity,
                            bias=bq_sb[:, oc:oc + 1])
                    else:
                        nc.scalar.activation(
                            t[:], q_ps[:], mybir.ActivationFunctionType.Copy)
                    qk_sb[(img, oc)] = t

            def emit_v(img, mcs):
                # V^T [m, c] per m-chunk; pairs packed for the DR attn@V.
                for mc in mcs:
                    v_ps = ps.tile([128, 512], F32, tag="s")
                    for u in range(2):
                        nc.tensor.matmul(
                            v_ps[:],
                            xm_r(img, u)[:, :, mc * 128:(mc + 1) * 128],
                            wq_r(u)[:, :, 2 * C:3 * C],
                            start=(u == 0), stop=(u == 1), perf_mode=DR,
                        )
                    j, slot = mc // 2, mc % 2
                    if slot == 0:
                        t = sb.tile([128, 2 * HEADS * 80], F8, tag="vt",
                                    bufs=BUFS["vt"], name=f"vt{img}_{j}")
                        tv = t[:].rearrange("p (i h u) -> p i h u", i=2, u=80)
                        nc.gpsimd.memset(tv[:, :, :, 64:65], 1.0)
                        vt_sb[(img, j)] = t
                    else:
                        t = vt_sb[(img, j)]
                        tv = t[:].rearrange("p (i h u) -> p i h u", i=2, u=80)
                    nc.scalar.activation(
                        tv[:, slot, :, 0:64],
                        v_ps[:].rearrange("p (h u) -> p h u", u=64),
                        mybir.ActivationFunctionType.Copy)

            def vt_head(img, h):
                out = []
                for j in range(4):
                    t = vt_sb[(img, j)][:].rearrange("p (i hu) -> p i hu", i=2)
                    out.append(t[:, :, h * 80:h * 80 + 65])
                return out

            def emit_head(img, h, phase, pos, filler=None):
                pair, half = h // 2, h % 2
                base = 64 * half
                qt, kt = qk_sb[(img, pair)], qk_sb[(img, 4 + pair)]
                pat = EXP_PAT[phase][pos]
                # O^T [n, c]: partitions = tokens of each 128-chunk, free =
                # [nc, 65] with the softmax denominator in column 64 (from
                # the ones column of vt). es is the stationary operand.
                o_ps = pso.tile([128, 8 * 65], F32, tag="o")
                opr = o_ps[:].rearrange("p (a u) -> p a u", u=65)
                es = {}

                def s_step(mc):
                    s_ps = ps.tile([128, N], F32, tag="s")
                    for nh in range(NH):
                        nc.tensor.matmul(
                            s_ps[:, nh * 512:(nh + 1) * 512],
                            kt[base:base + 64, mc * 128:(mc + 1) * 128],
                            qt[base:base + 64, nh * 512:(nh + 1) * 512],
                            start=True, stop=True,
                        )
                    j, slot = mc // 2, mc % 2
                    if slot == 0:
                        es[j] = sb.tile([128, 2 * N], F8, tag="es",
                                        bufs=BUFS["es"], name=f"es{j}")
                    ev = es[j][:, slot * N:(slot + 1) * N]
                    if pat[mc] == 'a':
                        nc.scalar.activation(
                            ev, s_ps[:], mybir.ActivationFunctionType.Exp,
                            scale=SCALE)
                    else:
                        nc.vector.tensor_scalar(
                            ev.bitcast(I8), s_ps[:], EXP_C1, EXP_C2,
                            mybir.AluOpType.mult, mybir.AluOpType.add)

                def o_chunks():
                    ers = [es[j][:].rearrange("p (i n) -> p i n", i=2)
                           for j in range(4)]
                    vh = vt_head(img, h)
                    for a in range(8):
                        for j in range(4):
                            nc.tensor.matmul(
                                opr[:, a, :],
                                ers[j][:, :, a * 128:(a + 1) * 128],
                                vh[j],
                                start=(j == 0), stop=(j == 3), perf_mode=DR,
                                skip_group_check=True,
                            )

                for mc in range(MC):
                    s_step(mc)
                    if mc == 1 and filler and len(filler) > 0 and filler[0]:
                        filler[0]()
                    if mc == 3 and filler and len(filler) > 1 and filler[1]:
                        filler[1]()
                    if mc == 5 and filler and len(filler) > 2 and filler[2]:
                        filler[2]()
                o_chunks()

                # normalize O^T by the denominator column into per-chunk
                # contiguous [128, 128] tiles (head pair interleaved in c);
                # dma_start_transpose needs contiguous src AND dst on HW
                if half == 0:
                    for a in range(8):
                        onT[(img, pair, a)] = sb.tile(
                            [128, 128], BF16, tag="onT", bufs=BUFS["onT"],
                            name=f"onT{img}_{pair}_{a}")
                if NORM_MODE == "div_psum":
                    for a in range(8):
                        nc.vector.tensor_scalar(
                            onT[(img, pair, a)][:, base:base + 64],
                            opr[:, a, 0:64],
                            opr[:, a:a + 1, 64:65], None,
                            mybir.AluOpType.divide)
                else:
                    dd = sb.tile([128, 8], F32, tag="dd", bufs=BUFS["dd"])
                    ddr = dd[:].rearrange("p (a u) -> p a u", u=1)
                    nc.vector.tensor_copy(ddr[:], opr[:, :, 64:65])
                    nc.vector.reciprocal(dd[:], dd[:])
                    for a in range(8):
                        nc.vector.tensor_scalar(
                            onT[(img, pair, a)][:, base:base + 64],
                            opr[:, a, 0:64],
                            dd[:, a:a + 1], None,
                            mybir.AluOpType.mult)

                if half == 1:
                    # pair complete: transpose [token, channel-pair] ->
                    # [channel, token] tiles feeding proj directly
                    for a in range(8):
                        t = tr[(img, pair, a)] = sb.tile(
                            [128, 128], BF16, tag="tr", bufs=BUFS["tr"],
                            name=f"tr{img}_{pair}_{a}")
                        nc.sync.dma_start_transpose(
                            out=t[:], in_=onT[(img, pair, a)][:])

            def emit_proj(img, ocs):
                for oc in ocs:
                    p_ps = ps.tile([128, N], F32, tag="s")
                    for a in range(8):
                        for cc in range(4):
                            nc.tensor.matmul(
                                p_ps[:, a * 128:(a + 1) * 128],
                                pw_dr[cc][:, oc * 128:(oc + 1) * 128],
                                tr[(img, cc, a)][:],
                                start=(cc == 0), stop=(cc == 3),
                            )
                    yt = sb.tile([128, N], F32, tag="y", bufs=BUFS["y"])
                    nc.vector.tensor_add(yt[:], p_ps[:], xr_sb[(img, oc)][:])
                    nc.sync.dma_start(out=y_h[img, oc * 128:(oc + 1) * 128, :],
                                      in_=yt[:])

            # ---------- emission schedule (2 images, pipelined) ----------
            # warm the ACT exp table during the input DMAs
            warm = wp.tile([1, 1], F32, tag="warm", name="warm")
            nc.vector.memset(warm[:], 0.0)
            nc.scalar.activation(
                warm[:], warm[:], mybir.ActivationFunctionType.Exp)
            bq_sb = load_weights()

            # minimal preamble: first head (h=1) needs Q/K chunk 0 and the
            # first V pair; the rest drains as fillers inside the head loop
            emit_qkv(0, [0, 4])
            emit_v(0, [0, 1, 2, 3])

            head_order = list(range(8))  # pair (2k, 2k+1) completes at the
            # odd head; its transpose DMAs fire there

            fillers0 = [
                [lambda: emit_qkv(0, [1]),
                 lambda: (emit_qkv(0, [5]), emit_v(0, [4, 5])),
                 lambda: emit_v(0, [6, 7])],
                [lambda: emit_qkv(0, [2]), lambda: emit_qkv(0, [6])],
                [lambda: emit_qkv(0, [3]),
                 lambda: emit_qkv(0, [7]),
                 lambda: load_xm(1)],
                [lambda: emit_qkv(1, [0]), lambda: emit_qkv(1, [4])],
                [lambda: emit_qkv(1, [1]),
                 lambda: emit_qkv(1, [5]),
                 lambda: emit_v(1, [0, 1])],
                [lambda: emit_qkv(1, [2]),
                 lambda: emit_qkv(1, [6]),
                 lambda: emit_v(1, [2, 3])],
                [lambda: emit_qkv(1, [3]),
                 lambda: emit_qkv(1, [7]),
                 lambda: emit_v(1, [4, 5])],
                [lambda: emit_v(1, [6, 7]),
                 lambda: load_xr(0)],
            ]
            for pos, h in enumerate(head_order):
                emit_head(0, h, 0, pos, filler=fillers0[pos])

            fillers1 = [
                None,
                [None, lambda: emit_proj(0, [0])],
                [None, lambda: emit_proj(0, [1])],
                [None, lambda: emit_proj(0, [2])],
                [None, lambda: emit_proj(0, [3])],
                [lambda: load_xr(1)],
                None,
                None,
            ]
            for pos, h in enumerate(head_order):
                emit_head(1, h, 1, pos, filler=fillers1[pos])
            emit_proj(1, range(4))

    _split_multi_waits(nc)
    return nc


_CACHE = {}


def _get_nc(mode=None):
    if "nc" not in _CACHE:
        _CACHE["nc"] = build_nc()
    return _CACHE["nc"]


def prepare_inputs(x, qkv_w, qkv_b, proj_w, proj_b):
    f8 = ml_dtypes.float8_e4m3
    x = np.asarray(x, np.float32).reshape(B, C, N)
    qkv_w = np.asarray(qkv_w, np.float32)
    qkv_b = np.asarray(qkv_b, np.float32)
    proj_w = np.asarray(proj_w, np.float32)
    proj_b = np.asarray(proj_b, np.float32)

    xm = np.ascontiguousarray(x.astype(f8))
    # residual with proj bias and the folded V-bias term (sum_m attn == 1)
    rbias = proj_b + proj_w.astype(ml_dtypes.bfloat16).astype(np.float32) @ qkv_b[2 * C:]
    xr = np.ascontiguousarray(x + rbias[None, :, None])
    wqkv = np.ascontiguousarray(qkv_w.T.astype(f8))
    pw = np.ascontiguousarray(proj_w.T.astype(ml_dtypes.bfloat16))
    bq = np.ascontiguousarray(qkv_b[:C].reshape(4, 128).T)

    in_maps = []
    for c in range(NCORES):
        sl = slice(c * BPC, (c + 1) * BPC)
        in_maps.append({
            "xm": xm[sl], "xr": xr[sl], "wqkv": wqkv, "pw": pw, "bq": bq,
        })
    return in_maps


def run(x, qkv_w, qkv_b, proj_w, proj_b, mode=None, **spmd_kwargs):
    nc = _get_nc()
    in_maps = prepare_inputs(x, qkv_w, qkv_b, proj_w, proj_b)
    res = run_bass_kernel_spmd(nc, in_maps, list(range(NCORES)), **spmd_kwargs)
    y = np.concatenate([np.asarray(res.results[c]["y"]) for c in range(NCORES)], axis=0)
    return res, y.reshape(B, C, 32, 32).astype(np.float32)


MM_MODE = "fp8dr"


def kernel(x, qkv_w, qkv_b, proj_w, proj_b):
    _, y = run(x, qkv_w, qkv_b, proj_w, proj_b)
    return y
